# revision 1
# baseline (speedup 1.0000x reference)
"""BitLinear (ternary weight) inference kernel for Trainium2, 8-core SPMD.

Full-input contract: kernel(**inputs) takes the complete tensors and returns
the complete output. The batch dim (B=8) is sharded 1:1 onto the 8
NeuronCores; each core computes y[b] = x[b] @ (w_q * 2^s_exp)^T + bias as a
2048^3 matmul (fp16 x, fp8 w, fp32 PSUM accumulation).

Host prep (cheap, O(bytes)): fold the power-of-two per-channel scale into
the ternary weights — values +-2^s / 0 are EXACT in fp8e4m3 — transpose
both operands into the PE's contraction-major [K, ...] layout, cast x to
fp16 (the only lossy step, ~2^-11 relative), broadcast bias to [128, OUT].

Device schedule (PE-bound; ~245us/core vs 218.5us matmul streaming floor):
  - Mixed-dtype matmuls: stationary x-tile fp16 [128,128], moving w fp8
    [128,512], one PSUM bank each, K accumulated 128 rows per step.
  - The first 6 row tiles run k-chunks 0..3 as soon as ~2 MiB of input has
    landed (pass A), parking partial sums in SBUF; the remaining k-chunks
    are added later (accum pass) interleaved with full-k single-pass tiles,
    so the PE never waits on the 12.6 MiB input stream.
  - Inputs on the Sync HWDGE ring, output stores on the Scalar HWDGE ring,
    epilogue (psum + bias / + partial -> SBUF) on the Vector engine, and a
    short dummy-matmul burst pre-warms the PE HAM clock gate.
"""
import os

import ml_dtypes
import numpy as np

B, T, IN, OUT = 8, 2048, 2048, 2048
P = 128
NCORES = 8
NF = 512        # matmul free dim (one PSUM bank of fp32)
KA = 4          # k-chunks in pass A (first-pass dependency set = KA MiB won't gate PE)

last_exec_time_ns = None
_CACHE = {}


def _install_prof_shim():
    """Make antenv.axon_hooks importable so trace=True works under axon."""
    import sys
    import types

    if "antenv.axon_hooks" in sys.modules:
        return
    try:
        from trn_agent_boot.trn_boot import _ntff_profile_via_ctypes
    except ImportError:
        return
    hook = _ntff_profile_via_ctypes("/opt/axon/libaxon_pjrt.so")
    mod = types.ModuleType("antenv.axon_hooks")
    mod.get_axon_ntff_profile_hook = lambda: hook
    mod.set_axon_ntff_profile_hook = lambda h: None
    sys.modules["antenv.axon_hooks"] = mod


def _build():
    import concourse.bacc as bacc
    import concourse.mybir as mybir
    from concourse.tile import TileContext

    nc = bacc.Bacc()
    x = nc.dram_tensor("x", (IN, T), mybir.dt.float16, kind="ExternalInput")
    w = nc.dram_tensor("w", (IN, OUT), mybir.dt.float8e4, kind="ExternalInput")
    bias = nc.dram_tensor("bias", (P, OUT), mybir.dt.float32, kind="ExternalInput")
    y = nc.dram_tensor("y", (T, OUT), mybir.dt.float32, kind="ExternalOutput")

    KT = IN // P    # contraction chunks
    TT = T // P     # output row tiles
    OC = OUT // NF  # psum banks per row tile

    HOUT = OUT // 2  # two psum tiles (2 banks each) per row tile

    with TileContext(nc) as tc:
        with tc.tile_pool(name="wp", bufs=1) as wp, \
             tc.tile_pool(name="xp", bufs=1) as xp, \
             tc.tile_pool(name="bp", bufs=1) as bp, \
             tc.tile_pool(name="op", bufs=4) as op_, \
             tc.tile_pool(name="ptp", bufs=1) as ptp, \
             tc.tile_pool(name="pp", bufs=4, space="PSUM") as pp:

            # Interleave w/x chunk loads k-wise so pass A's working set
            # (k < KA) lands first and the PE can start after ~2 MiB.
            # Later chunks load pairwise (>=1 MiB DMAs for efficiency).
            w_tiles = [None] * KT
            xT_tiles = [None] * KT
            bias_t = bp.tile([P, OUT], mybir.dt.float32, tag="bias")
            x3 = x.rearrange("(ko p) t -> p ko t", p=P)
            w3 = w.rearrange("(ko p) o -> p ko o", p=P)

            # HAM pre-warm: a short burst of dummy matmuls on a scratch tile
            # while the first loads are in flight, so the PE clock-gate is
            # near 8/8 when the real matmuls start. Uses one "ps" slot
            # briefly (released well before pass A needs its 4th buffer).
            warm_sb = bp.tile([P, NF], mybir.dt.float16, tag="warm")
            nc.gpsimd.memset(warm_sb, 0.0)
            warm_ps = pp.tile([P, HOUT], mybir.dt.float32, tag="ps",
                              name="warmps")
            for i in range(6):
                nc.tensor.matmul(warm_ps[:, :NF], warm_sb[:, :P], warm_sb,
                                 start=(i == 0), stop=(i == 5))

            HT = T // 2
            for k in range(KA):
                wt = wp.tile([P, OUT], mybir.dt.float8e4, tag=f"w{k}")
                xt = xp.tile([P, T], mybir.dt.float16, tag=f"x{k}")
                nc.sync.dma_start(wt, w[k * P:(k + 1) * P, :])
                nc.sync.dma_start(xt[:, :HT], x[k * P:(k + 1) * P, :HT])
                w_tiles[k] = wt
                xT_tiles[k] = xt
            nc.sync.dma_start(bias_t, bias[:, :])
            for k in range(KA, KT, 2):
                wt2 = wp.tile([P, 2, OUT], mybir.dt.float8e4, tag=f"w{k}")
                nc.sync.dma_start(wt2, w3[:, k:k + 2, :])
                w_tiles[k] = wt2[:, 0]
                w_tiles[k + 1] = wt2[:, 1]
                xt2 = xp.tile([P, 2, T], mybir.dt.float16, tag=f"x{k}")
                nc.sync.dma_start(xt2, x3[:, k:k + 2, :])
                xT_tiles[k] = xt2[:, 0]
                xT_tiles[k + 1] = xt2[:, 1]
            # deferred: t>=1024 halves of the pass-A x chunks are only read
            # by single-pass row tiles 8+, which run ~50us after this lands
            for k in range(KA):
                nc.sync.dma_start(xT_tiles[k][:, HT:], x[k * P:(k + 1) * P, HT:])

            TSPLIT = 6       # row tiles 0..TSPLIT-1 two-pass (partials in SBUF)

            partial_tiles = [
                ptp.tile([P, OUT], mybir.dt.float32, tag=f"pt{j}", name=f"pt{j}")
                for j in range(TSPLIT)
            ]

            def do_tiles(tt_range, k_lo, k_hi, mode):
                # mode: "partial" = bias add into SBUF partial (no store),
                #       "accum" = add SBUF partial + store,
                #       "single" = bias add + store
                for tt in tt_range:
                    pss = [pp.tile([P, HOUT], mybir.dt.float32, tag="ps",
                                   name=f"ps{h}") for h in range(2)]
                    for k in range(k_lo, k_hi):
                        lhsT = xT_tiles[k][:, tt * P:(tt + 1) * P]
                        for oc in range(OC):
                            ps = pss[oc // 2]
                            lo = (oc % 2) * NF
                            nc.tensor.matmul(
                                ps[:, lo:lo + NF],
                                lhsT,
                                w_tiles[k][:, oc * NF:(oc + 1) * NF],
                                start=(k == k_lo),
                                stop=(k == k_hi - 1),
                            )
                    if mode == "partial":
                        ot = partial_tiles[tt]
                    else:
                        ot = op_.tile([P, OUT], mybir.dt.float32, tag="out")
                    if tt == TT - 1:
                        # last tile: chunk epilogue+store so the store of
                        # chunk q overlaps the add of chunk q+1 (short tail)
                        for q in range(OC):
                            sl = slice(q * NF, (q + 1) * NF)
                            psl = slice((q % 2) * NF, (q % 2) * NF + NF)
                            nc.vector.tensor_add(ot[:, sl], pss[q // 2][:, psl],
                                                 bias_t[:, sl])
                            eng = nc.scalar if q % 2 == 0 else nc.sync
                            eng.dma_start(y[tt * P:(tt + 1) * P, sl],
                                          ot[:, sl])
                        continue
                    for h in range(2):
                        sl = slice(h * HOUT, (h + 1) * HOUT)
                        if mode == "accum":
                            nc.vector.tensor_add(ot[:, sl], pss[h],
                                                 partial_tiles[tt][:, sl])
                        else:
                            nc.vector.tensor_add(ot[:, sl], pss[h], bias_t[:, sl])
                    if mode != "partial":
                        nc.scalar.dma_start(y[tt * P:(tt + 1) * P, :], ot)

            do_tiles(range(TSPLIT), 0, KA, "partial")
            # Interleave accum and single-pass tiles so the PE always has
            # runnable chunks while the tail of the input load streams in.
            for j in range(TT - TSPLIT):
                if j < TSPLIT:
                    do_tiles([j], KA, KT, "accum")
                do_tiles([TSPLIT + j], 0, KT, "single")

    nc.compile()
    return nc


def kernel(x, w_q, s_exp, bias):
    global last_exec_time_ns
    from concourse.bass_utils import run_bass_kernel_spmd

    x = np.asarray(x)
    w_q = np.asarray(w_q)
    s_exp = np.asarray(s_exp)
    bias = np.asarray(bias, dtype=np.float32)
    assert x.shape == (B, T, IN) and w_q.shape == (OUT, IN)

    # Fold the power-of-two per-output-channel scale into the ternary
    # weights: values are +-2^s or 0 with s in [-8, 0], exact in fp8e4m3
    # (2^-8 and 2^-9 are exact subnormals).
    scale = np.exp2(s_exp.astype(np.float32))
    w_scaled_t = (w_q.astype(np.float32) * scale[:, None]).T
    w_fp8 = np.ascontiguousarray(w_scaled_t).astype(ml_dtypes.float8_e4m3fn)
    if not np.array_equal(w_fp8.astype(np.float32), w_scaled_t):
        import warnings
        warnings.warn("scaled ternary weights not exact in fp8e4m3; "
                      "proceeding with rounded weights")
    bias_bcast = np.ascontiguousarray(
        np.broadcast_to(bias.astype(np.float32), (P, OUT)))
    # Contraction-major layout for the PE: x^T[b] = [IN, T], fp16.
    xT_f16 = np.ascontiguousarray(
        x.astype(np.float16).transpose(0, 2, 1))

    nc = _CACHE.get("nc")
    if nc is None:
        nc = _CACHE["nc"] = _build()

    in_maps = [
        {"x": xT_f16[b], "w": w_fp8, "bias": bias_bcast} for b in range(B)
    ]

    trace = bool(int(os.environ.get("BITLIN_TRACE", "0")))
    if trace:
        _install_prof_shim()
    res = run_bass_kernel_spmd(nc, in_maps, list(range(NCORES)), trace=trace)
    last_exec_time_ns = res.exec_time_ns

    out = np.stack([res.results[b]["y"] for b in range(B)], axis=0)
    return out.astype(np.float32, copy=False)



# revision 3
# speedup vs baseline: 1.0240x; 1.0240x over previous
"""BitLinear (ternary weight) inference kernel for Trainium2, 8-core SPMD.

Full-input contract: kernel(**inputs) takes the complete tensors and returns
the complete output. The batch dim (B=8) is sharded 1:1 onto the 8
NeuronCores; each core computes y[b] = x[b] @ (w_q * 2^s_exp)^T + bias as a
2048^3 matmul.

Split-precision scheme: output channels are sorted by s_exp (host-side
permutation of w/bias columns, inverted on the output). The 2^s scale is
folded into the ternary weights (+-2^s / 0 are EXACT in fp8e4m3). Channels
with small 2^s (the first RA=1792 after sorting) tolerate fp8 activations:
they use fp8e4m3 x with perf_mode=DoubleRow (2 fp8 weights/cell -> 2
MACs/cell/cycle). The top RB=256 channels (all of s=0) keep fp16
activations at the regular rate. Measured rel err of this split on the
reference data: ~1.11e-2 (gate 2e-2).

Device schedule per core:
  - All inputs stream on the Sync HWDGE ring (a single queue already fans
    across all 16 SDMA engines, ~400 GB/s): x8/w k-pair-interleaved so the
    contraction frontier advances every ~2.2 us, then the fp16-path data.
    Outputs go on the Scalar ring, bias broadcast-loads on the SWDGE queue.
  - Phase DR (cols 0..1792): 16 row tiles, each a full-K chain of
    DoubleRow matmuls (lhsT [128,2,128] fp8 x pairs, rhs [128,2,*] fp8 w),
    3x512-col psum + 1x256-col psum per tile, two tiles in flight.
  - Phase F16 (cols 1792..2048): fp16 x [128,128] stationary, fp8 w
    [128,256] moving, 16 k-steps per row tile.
  - Epilogue on DVE (psum + bias -> fp16 SBUF), dummy-matmul PE warmup
    sized so the HAM clock gate is at 8/8 when real data lands.
"""
import os

import ml_dtypes
import numpy as np

B, T, IN, OUT = 8, 2048, 2048, 2048
P = 128
NCORES = 8
NF = 512        # psum bank of fp32 / max matmul free dim
RB = 256        # fp16-path columns (largest-s channels, after sorting)
RA = OUT - RB   # DoubleRow fp8 columns (1792)
KP = IN // (2 * P)   # 8 k-pairs
KQ = IN // (4 * P)   # 4 k-quads (x16 DMA granularity)

last_exec_time_ns = None
_CACHE = {}


def _install_prof_shim():
    """Make antenv.axon_hooks importable so trace=True works under axon."""
    import sys
    import types

    if "antenv.axon_hooks" in sys.modules:
        return
    try:
        from trn_agent_boot.trn_boot import _ntff_profile_via_ctypes
    except ImportError:
        return
    hook = _ntff_profile_via_ctypes("/opt/axon/libaxon_pjrt.so")
    mod = types.ModuleType("antenv.axon_hooks")
    mod.get_axon_ntff_profile_hook = lambda: hook
    mod.set_axon_ntff_profile_hook = lambda h: None
    sys.modules["antenv.axon_hooks"] = mod


def _build():
    import concourse.bacc as bacc
    import concourse.bass as bass
    import concourse.mybir as mybir
    from concourse.tile import TileContext

    DR = mybir.MatmulPerfMode.DoubleRow

    nc = bacc.Bacc()
    x8 = nc.dram_tensor("x8", (IN, T), mybir.dt.float8e4, kind="ExternalInput")
    x16 = nc.dram_tensor("x16", (IN, T), mybir.dt.float16, kind="ExternalInput")
    w = nc.dram_tensor("w", (IN, OUT), mybir.dt.float8e4, kind="ExternalInput")
    bias = nc.dram_tensor("bias", (OUT,), mybir.dt.float32, kind="ExternalInput")
    y = nc.dram_tensor("y", (T, OUT), mybir.dt.float16, kind="ExternalOutput")

    TT = T // P     # output row tiles

    with TileContext(nc) as tc:
        with tc.tile_pool(name="wap", bufs=1) as wap, \
             tc.tile_pool(name="x8p", bufs=1) as x8p, \
             tc.tile_pool(name="x16p", bufs=1) as x16p, \
             tc.tile_pool(name="bp", bufs=1) as bp, \
             tc.tile_pool(name="opa", bufs=4) as opa, \
             tc.tile_pool(name="opb", bufs=4) as opb, \
             tc.tile_pool(name="pp", bufs=2, space="PSUM") as pp, \
             tc.tile_pool(name="pg", bufs=2, space="PSUM") as pg:

            bias_t = bp.tile([P, OUT], mybir.dt.float32, tag="bias")
            # Broadcast-load bias (8 KB in DRAM) to all 128 partitions via
            # the SWDGE queue: leading stride-0 AP dim replicates it. Keeps
            # the broadcast off the critical HWDGE input stream.
            bias_ap = bias[:]
            bias_bc = bass.AP(
                tensor=bias_ap.tensor,
                offset=bias_ap.offset,
                ap=[[0, P]] + list(bias_ap.ap),
            )
            nc.gpsimd.dma_start(out=bias_t, in_=bias_bc)

            # HAM pre-warm: dummy matmuls on uninitialized SBUF (the psum
            # target is never read), sized to keep the PE busy until real
            # data arrives so the clock gate reaches 8/8 and stays there.
            warm_sb = bp.tile([P, NF], mybir.dt.float16, tag="warm")
            nc.vector.memset(warm_sb, 0.0)
            warm_ps = pg.tile([P, NF], mybir.dt.float32, tag="psB",
                              name="warmps")
            NWARM = 22
            for i in range(NWARM):
                nc.tensor.matmul(warm_ps, warm_sb[:, :P], warm_sb,
                                 start=(i == 0), stop=(i == NWARM - 1))

            # DRAM views with the k-chunk index explicit.
            x83 = x8.rearrange("(ko p) t -> p ko t", p=P)
            x163 = x16.rearrange("(ko p) t -> p ko t", p=P)
            w3 = w.rearrange("(ko p) o -> p ko o", p=P)

            # Input loads, all on the sync ring in consumption order. The
            # early stream is one token-quarter of each x8 k-pair plus the
            # full (contiguous) w k-pair — ~0.62 MiB per k-pair, matching
            # the two-in-flight row tiles' consumption rate, so the PE
            # tracks the input frontier with ~95% duty. The remaining x8
            # token-quarters, then x16, ride behind.
            x8_t = []
            wa_t = []
            TQ = T // 4
            for kp in range(KP):
                xt = x8p.tile([P, 2, T], mybir.dt.float8e4, tag=f"x8_{kp}")
                wt = wap.tile([P, 2, OUT], mybir.dt.float8e4, tag=f"wa{kp}")
                if kp == 0:
                    # first k-pair: column-quartered w so the first matmul
                    # only waits on ~0.25 MiB
                    nc.sync.dma_start(xt[:, :, :TQ],
                                      x83[:, 0:2, :TQ])
                    for oq in range(4):
                        osl = slice(oq * NF, (oq + 1) * NF)
                        nc.sync.dma_start(wt[:, :, osl], w3[:, 0:2, osl])
                else:
                    nc.sync.dma_start(wt, w3[:, 2 * kp:2 * kp + 2, :])
                    nc.sync.dma_start(xt[:, :, :TQ],
                                      x83[:, 2 * kp:2 * kp + 2, :TQ])
                x8_t.append(xt)
                wa_t.append(wt)
            # remaining token-quarters; quarter 1 is needed by row tile 4
            # at ~30 us, so it goes in one (larger, efficient) load per
            # half-k, ahead of quarters 2-3
            for tq in range(1, 4):
                qsl = slice(tq * TQ, (tq + 1) * TQ)
                for kp in range(KP):
                    nc.sync.dma_start(x8_t[kp][:, :, qsl],
                                      x83[:, 2 * kp:2 * kp + 2, qsl])
            x16_t = []
            for q in range(KQ):
                xt = x16p.tile([P, 4, T], mybir.dt.float16, tag=f"x16_{q}")
                nc.sync.dma_start(xt, x163[:, 4 * q:4 * q + 4, :])
                x16_t.append(xt)

            # Phase DR: 16 full-K row tiles, 1792 cols each.
            for tt in range(TT):
                tsl = slice(tt * P, (tt + 1) * P)
                ps = pp.tile([P, 3 * NF], mybir.dt.float32, tag="psA")
                pq = pg.tile([P, NF], mybir.dt.float32, tag="psB")
                for kp in range(KP):
                    lhsT = x8_t[kp][:, :, tsl]
                    for oc in range(3):
                        nc.tensor.matmul(
                            ps[:, oc * NF:(oc + 1) * NF],
                            lhsT,
                            wa_t[kp][:, :, oc * NF:(oc + 1) * NF],
                            start=(kp == 0), stop=(kp == KP - 1),
                            perf_mode=DR,
                        )
                    nc.tensor.matmul(
                        pq[:, :RA - 3 * NF],
                        lhsT,
                        wa_t[kp][:, :, 3 * NF:RA],
                        start=(kp == 0), stop=(kp == KP - 1),
                        perf_mode=DR,
                    )
                ot = opa.tile([P, RA], mybir.dt.float16, tag="outA")
                nc.vector.tensor_add(ot[:, :3 * NF], ps, bias_t[:, :3 * NF])
                nc.vector.tensor_add(ot[:, 3 * NF:], pq[:, :RA - 3 * NF],
                                     bias_t[:, 3 * NF:RA])
                nc.scalar.dma_start(y[tt * P:(tt + 1) * P, :RA], ot)

            # Phase F16 (cols RA..OUT)
            for tt in range(TT):
                tsl = slice(tt * P, (tt + 1) * P)
                pq = pg.tile([P, NF], mybir.dt.float32, tag="psB")
                for k in range(IN // P):
                    q, i = divmod(k, 4)
                    nc.tensor.matmul(
                        pq[:, :RB],
                        x16_t[q][:, i, tsl],
                        wa_t[k // 2][:, k % 2, RA:],
                        start=(k == 0),
                        stop=(k == IN // P - 1),
                    )
                ot = opb.tile([P, RB], mybir.dt.float16, tag="outB")
                if tt == TT - 1:
                    # last tile: chunk epilogue+store to shorten the tail
                    c = RB // 2
                    for h in range(2):
                        sl = slice(h * c, (h + 1) * c)
                        nc.vector.tensor_add(ot[:, sl], pq[:, sl],
                                             bias_t[:, RA + h * c:
                                                    RA + (h + 1) * c])
                        eng = nc.scalar if h == 0 else nc.sync
                        eng.dma_start(y[tt * P:(tt + 1) * P,
                                        RA + h * c:RA + (h + 1) * c],
                                      ot[:, sl])
                else:
                    nc.vector.tensor_add(ot, pq[:, :RB], bias_t[:, RA:])
                    nc.scalar.dma_start(y[tt * P:(tt + 1) * P, RA:], ot)

    nc.compile()
    return nc


def kernel(x, w_q, s_exp, bias):
    global last_exec_time_ns
    from concourse.bass_utils import run_bass_kernel_spmd

    x = np.asarray(x)
    w_q = np.asarray(w_q)
    s_exp = np.asarray(s_exp)
    bias = np.asarray(bias, dtype=np.float32)
    assert x.shape == (B, T, IN) and w_q.shape == (OUT, IN)

    # Sort output channels by s_exp ascending: the first RA (small 2^s)
    # go through the fp8 DoubleRow path, the top RB through fp16.
    perm = np.argsort(s_exp, kind="stable")

    # Fold the power-of-two per-output-channel scale into the ternary
    # weights: values are +-2^s or 0 with s in [-8, 0], exact in fp8e4m3
    # (2^-8 and 2^-9 are exact subnormals).
    scale = np.exp2(s_exp.astype(np.float32))
    w_scaled_t = (w_q.astype(np.float32) * scale[:, None])[perm].T
    w_fp8 = np.ascontiguousarray(w_scaled_t).astype(ml_dtypes.float8_e4m3fn)
    if not np.array_equal(w_fp8.astype(np.float32), w_scaled_t):
        import warnings
        warnings.warn("scaled ternary weights not exact in fp8e4m3; "
                      "proceeding with rounded weights")
    bias_perm = np.ascontiguousarray(bias[perm].astype(np.float32))
    # Contraction-major layouts for the PE: x^T[b] = [IN, T].
    xT = x.transpose(0, 2, 1)
    xT_f16 = np.ascontiguousarray(xT.astype(np.float16))
    xT_f8 = np.ascontiguousarray(xT.astype(ml_dtypes.float8_e4m3fn))

    nc = _CACHE.get("nc")
    if nc is None:
        nc = _CACHE["nc"] = _build()

    in_maps = [
        {"x8": xT_f8[b], "x16": xT_f16[b], "w": w_fp8, "bias": bias_perm}
        for b in range(B)
    ]

    trace = bool(int(os.environ.get("BITLIN_TRACE", "0")))
    if trace:
        _install_prof_shim()
    res = run_bass_kernel_spmd(nc, in_maps, list(range(NCORES)), trace=trace)
    last_exec_time_ns = res.exec_time_ns
    _CACHE["res"] = res

    y_dev = np.stack([np.asarray(res.results[b]["y"]) for b in range(B)],
                     axis=0).astype(np.float32)
    out = np.empty((B, T, OUT), dtype=np.float32)
    out[:, :, perm] = y_dev

    # Safety net: every channel at the maximum scale 2^smax must take the
    # fp16 path (fp8 error there is ~2.4e-2 > the 2e-2 gate). The device
    # fp16 region holds RB=256 channels; with the spec's s distribution
    # (~228 at smax) it always fits, but if an unusual draw overflows it,
    # recompute the spilled channels exactly on the host.
    smax = int(s_exp.max())
    spilled = perm[:RA][s_exp[perm[:RA]] == smax]
    if spilled.size:
        w_sp = (w_q[spilled].astype(np.float32)
                * np.exp2(s_exp[spilled].astype(np.float32))[:, None])
        out[:, :, spilled] = (
            np.einsum("bti,oi->bto", x.astype(np.float32), w_sp,
                      optimize=True) + bias[spilled])
    return out


# revision 4
# speedup vs baseline: 1.0681x; 1.0431x over previous
"""BitLinear (ternary weight) inference kernel for Trainium2, 8-core SPMD.

Full-input contract: kernel(**inputs) takes the complete tensors and returns
the complete output. The batch dim (B=8) is sharded 1:1 onto the 8
NeuronCores; each core computes y[b] = x[b] @ (w_q * 2^s_exp)^T + bias as a
2048^3 matmul.

Split-precision scheme: output channels are sorted by s_exp (host-side
permutation of w/bias columns, inverted on the output). The 2^s scale is
folded into the ternary weights (+-2^s / 0 are EXACT in fp8e4m3). Channels
with small 2^s (the first RA=1792 after sorting) tolerate fp8 activations:
they use fp8e4m3 x with perf_mode=DoubleRow (2 fp8 weights/cell -> 2
MACs/cell/cycle). The top RB=256 channels (all of s=0) keep fp16
activations at the regular rate for K >= 512. Measured rel err of this
split on the reference data: ~1.18e-2 (gate 2e-2).

Device schedule per core:
  - All inputs stream on the Sync HWDGE ring (a single queue already fans
    across all 16 SDMA engines, ~400 GB/s): x8/w k-pair-interleaved so the
    contraction frontier advances every ~2.2 us, then the fp16-path data.
    Outputs go on the Scalar ring, bias broadcast-loads on the SWDGE queue.
  - Phase DR (cols 0..1792): 16 row tiles, each a full-K chain of
    DoubleRow matmuls (lhsT [128,2,128] fp8 x pairs, rhs [128,2,*] fp8 w),
    3x512-col psum + 1x256-col psum per tile, two tiles in flight.
  - Phase top-channels (cols 1792..2048): K 0..512 via fp8 DoubleRow on
    the resident x8 (error sqrt(512/2048) of full-fp8), K 512..2048 via
    fp16 x [128,128] stationary x fp8 w [128,256] moving. The x16 quad 0
    stays loaded even though unread — removing it shifts the SBUF layout
    and measurably slows every matmul (~15%).
  - Epilogue on DVE (psum + bias -> fp16 SBUF), dummy-matmul PE warmup
    sized so the HAM clock gate is at 8/8 when real data lands.
"""
import os

import ml_dtypes
import numpy as np

B, T, IN, OUT = 8, 2048, 2048, 2048
P = 128
NCORES = 8
NF = 512        # psum bank of fp32 / max matmul free dim
RB = 256        # fp16-path columns (largest-s channels, after sorting)
RA = OUT - RB   # DoubleRow fp8 columns (1792)
KP = IN // (2 * P)   # 8 k-pairs
KQ = IN // (4 * P)   # 4 k-quads (x16 DMA granularity)

last_exec_time_ns = None
_CACHE = {}


def _install_prof_shim():
    """Make antenv.axon_hooks importable so trace=True works under axon."""
    import sys
    import types

    if "antenv.axon_hooks" in sys.modules:
        return
    try:
        from trn_agent_boot.trn_boot import _ntff_profile_via_ctypes
    except ImportError:
        return
    hook = _ntff_profile_via_ctypes("/opt/axon/libaxon_pjrt.so")
    mod = types.ModuleType("antenv.axon_hooks")
    mod.get_axon_ntff_profile_hook = lambda: hook
    mod.set_axon_ntff_profile_hook = lambda h: None
    sys.modules["antenv.axon_hooks"] = mod


def _build():
    import concourse.bacc as bacc
    import concourse.bass as bass
    import concourse.mybir as mybir
    from concourse.tile import TileContext

    DR = mybir.MatmulPerfMode.DoubleRow

    nc = bacc.Bacc()
    x8 = nc.dram_tensor("x8", (IN, T), mybir.dt.float8e4, kind="ExternalInput")
    x16 = nc.dram_tensor("x16", (IN, T), mybir.dt.float16, kind="ExternalInput")
    w = nc.dram_tensor("w", (IN, OUT), mybir.dt.float8e4, kind="ExternalInput")
    bias = nc.dram_tensor("bias", (OUT,), mybir.dt.float32, kind="ExternalInput")
    y = nc.dram_tensor("y", (T, OUT), mybir.dt.float16, kind="ExternalOutput")

    TT = T // P     # output row tiles

    with TileContext(nc) as tc:
        with tc.tile_pool(name="wap", bufs=1) as wap, \
             tc.tile_pool(name="x8p", bufs=1) as x8p, \
             tc.tile_pool(name="x16p", bufs=1) as x16p, \
             tc.tile_pool(name="bp", bufs=1) as bp, \
             tc.tile_pool(name="opa", bufs=4) as opa, \
             tc.tile_pool(name="opb", bufs=4) as opb, \
             tc.tile_pool(name="pp", bufs=2, space="PSUM") as pp, \
             tc.tile_pool(name="pg", bufs=2, space="PSUM") as pg:

            bias_t = bp.tile([P, OUT], mybir.dt.float32, tag="bias")
            # Broadcast-load bias (8 KB in DRAM) to all 128 partitions via
            # the SWDGE queue: leading stride-0 AP dim replicates it. Keeps
            # the broadcast off the critical HWDGE input stream.
            bias_ap = bias[:]
            bias_bc = bass.AP(
                tensor=bias_ap.tensor,
                offset=bias_ap.offset,
                ap=[[0, P]] + list(bias_ap.ap),
            )
            nc.gpsimd.dma_start(out=bias_t, in_=bias_bc)

            # HAM pre-warm: dummy matmuls on uninitialized SBUF (the psum
            # target is never read), sized to keep the PE busy until real
            # data arrives so the clock gate reaches 8/8 and stays there.
            warm_sb = bp.tile([P, NF], mybir.dt.float16, tag="warm")
            nc.vector.memset(warm_sb, 0.0)
            warm_ps = pg.tile([P, NF], mybir.dt.float32, tag="psB",
                              name="warmps")
            NWARM = 22
            for i in range(NWARM):
                nc.tensor.matmul(warm_ps, warm_sb[:, :P], warm_sb,
                                 start=(i == 0), stop=(i == NWARM - 1))

            # DRAM views with the k-chunk index explicit.
            x83 = x8.rearrange("(ko p) t -> p ko t", p=P)
            x163 = x16.rearrange("(ko p) t -> p ko t", p=P)
            w3 = w.rearrange("(ko p) o -> p ko o", p=P)

            # Input loads, all on the sync ring in consumption order. The
            # early stream is one token-quarter of each x8 k-pair plus the
            # full (contiguous) w k-pair — ~0.62 MiB per k-pair, matching
            # the two-in-flight row tiles' consumption rate, so the PE
            # tracks the input frontier with ~95% duty. The remaining x8
            # token-quarters, then x16, ride behind.
            x8_t = []
            wa_t = []
            TQ = T // 4
            for kp in range(KP):
                xt = x8p.tile([P, 2, T], mybir.dt.float8e4, tag=f"x8_{kp}")
                wt = wap.tile([P, 2, OUT], mybir.dt.float8e4, tag=f"wa{kp}")
                if kp == 0:
                    # first k-pair: column-quartered w so the first matmul
                    # only waits on ~0.25 MiB
                    nc.sync.dma_start(xt[:, :, :TQ],
                                      x83[:, 0:2, :TQ])
                    for oq in range(4):
                        osl = slice(oq * NF, (oq + 1) * NF)
                        nc.sync.dma_start(wt[:, :, osl], w3[:, 0:2, osl])
                else:
                    nc.sync.dma_start(wt, w3[:, 2 * kp:2 * kp + 2, :])
                    nc.sync.dma_start(xt[:, :, :TQ],
                                      x83[:, 2 * kp:2 * kp + 2, :TQ])
                x8_t.append(xt)
                wa_t.append(wt)
            # remaining token-quarters; quarter 1 is needed by row tile 4
            # at ~30 us, so it goes in one (larger, efficient) load per
            # half-k, ahead of quarters 2-3
            for tq in range(1, 4):
                qsl = slice(tq * TQ, (tq + 1) * TQ)
                for kp in range(KP):
                    nc.sync.dma_start(x8_t[kp][:, :, qsl],
                                      x83[:, 2 * kp:2 * kp + 2, qsl])
            x16_t = []
            for q in range(KQ):
                xt = x16p.tile([P, 4, T], mybir.dt.float16, tag=f"x16_{q}")
                nc.sync.dma_start(xt, x163[:, 4 * q:4 * q + 4, :])
                x16_t.append(xt)

            # Phase DR: 16 full-K row tiles, 1792 cols each.
            for tt in range(TT):
                tsl = slice(tt * P, (tt + 1) * P)
                ps = pp.tile([P, 3 * NF], mybir.dt.float32, tag="psA")
                pq = pg.tile([P, NF], mybir.dt.float32, tag="psB")
                for kp in range(KP):
                    lhsT = x8_t[kp][:, :, tsl]
                    for oc in range(3):
                        nc.tensor.matmul(
                            ps[:, oc * NF:(oc + 1) * NF],
                            lhsT,
                            wa_t[kp][:, :, oc * NF:(oc + 1) * NF],
                            start=(kp == 0), stop=(kp == KP - 1),
                            perf_mode=DR,
                        )
                    nc.tensor.matmul(
                        pq[:, :RA - 3 * NF],
                        lhsT,
                        wa_t[kp][:, :, 3 * NF:RA],
                        start=(kp == 0), stop=(kp == KP - 1),
                        perf_mode=DR,
                    )
                ot = opa.tile([P, RA], mybir.dt.float16, tag="outA")
                nc.vector.tensor_add(ot[:, :3 * NF], ps, bias_t[:, :3 * NF])
                nc.vector.tensor_add(ot[:, 3 * NF:], pq[:, :RA - 3 * NF],
                                     bias_t[:, 3 * NF:RA])
                nc.scalar.dma_start(y[tt * P:(tt + 1) * P, :RA], ot)

            # Phase top-channels (cols RA..OUT): K 0..512 in fp8 DoubleRow
            # (x8 is resident; error there is ~sqrt(512/2048) of full-fp8,
            # measured 1.18e-2 combined), K 512..2048 in fp16.
            for tt in range(TT):
                tsl = slice(tt * P, (tt + 1) * P)
                pq = pg.tile([P, NF], mybir.dt.float32, tag="psB")
                for kp in range(2):
                    nc.tensor.matmul(
                        pq[:, :RB],
                        x8_t[kp][:, :, tsl],
                        wa_t[kp][:, :, RA:],
                        start=(kp == 0), stop=False,
                        perf_mode=DR,
                    )
                for k in range(4, IN // P):
                    q, i = divmod(k, 4)
                    nc.tensor.matmul(
                        pq[:, :RB],
                        x16_t[q][:, i, tsl],
                        wa_t[k // 2][:, k % 2, RA:],
                        start=False,
                        stop=(k == IN // P - 1),
                    )
                ot = opb.tile([P, RB], mybir.dt.float16, tag="outB")
                if tt == TT - 1:
                    # last tile: chunk epilogue+store to shorten the tail
                    c = RB // 2
                    for h in range(2):
                        sl = slice(h * c, (h + 1) * c)
                        nc.vector.tensor_add(ot[:, sl], pq[:, sl],
                                             bias_t[:, RA + h * c:
                                                    RA + (h + 1) * c])
                        eng = nc.scalar if h == 0 else nc.sync
                        eng.dma_start(y[tt * P:(tt + 1) * P,
                                        RA + h * c:RA + (h + 1) * c],
                                      ot[:, sl])
                else:
                    nc.vector.tensor_add(ot, pq[:, :RB], bias_t[:, RA:])
                    nc.scalar.dma_start(y[tt * P:(tt + 1) * P, RA:], ot)

    nc.compile()
    return nc


def kernel(x, w_q, s_exp, bias):
    global last_exec_time_ns
    from concourse.bass_utils import run_bass_kernel_spmd

    x = np.asarray(x)
    w_q = np.asarray(w_q)
    s_exp = np.asarray(s_exp)
    bias = np.asarray(bias, dtype=np.float32)
    assert x.shape == (B, T, IN) and w_q.shape == (OUT, IN)

    # Sort output channels by s_exp ascending: the first RA (small 2^s)
    # go through the fp8 DoubleRow path, the top RB through fp16.
    perm = np.argsort(s_exp, kind="stable")

    # Fold the power-of-two per-output-channel scale into the ternary
    # weights: values are +-2^s or 0 with s in [-8, 0], exact in fp8e4m3
    # (2^-8 and 2^-9 are exact subnormals).
    scale = np.exp2(s_exp.astype(np.float32))
    w_scaled_t = (w_q.astype(np.float32) * scale[:, None])[perm].T
    w_fp8 = np.ascontiguousarray(w_scaled_t).astype(ml_dtypes.float8_e4m3fn)
    if not np.array_equal(w_fp8.astype(np.float32), w_scaled_t):
        import warnings
        warnings.warn("scaled ternary weights not exact in fp8e4m3; "
                      "proceeding with rounded weights")
    bias_perm = np.ascontiguousarray(bias[perm].astype(np.float32))
    # Contraction-major layouts for the PE: x^T[b] = [IN, T].
    xT = x.transpose(0, 2, 1)
    xT_f16 = np.ascontiguousarray(xT.astype(np.float16))
    xT_f8 = np.ascontiguousarray(xT.astype(ml_dtypes.float8_e4m3fn))

    nc = _CACHE.get("nc")
    if nc is None:
        nc = _CACHE["nc"] = _build()

    in_maps = [
        {"x8": xT_f8[b], "x16": xT_f16[b], "w": w_fp8, "bias": bias_perm}
        for b in range(B)
    ]

    trace = bool(int(os.environ.get("BITLIN_TRACE", "0")))
    if trace:
        _install_prof_shim()
    res = run_bass_kernel_spmd(nc, in_maps, list(range(NCORES)), trace=trace)
    last_exec_time_ns = res.exec_time_ns
    _CACHE["res"] = res

    y_dev = np.stack([np.asarray(res.results[b]["y"]) for b in range(B)],
                     axis=0).astype(np.float32)
    out = np.empty((B, T, OUT), dtype=np.float32)
    out[:, :, perm] = y_dev

    # Safety net: every channel at the maximum scale 2^smax must take the
    # fp16 path (fp8 error there is ~2.4e-2 > the 2e-2 gate). The device
    # fp16 region holds RB=256 channels; with the spec's s distribution
    # (~228 at smax) it always fits, but if an unusual draw overflows it,
    # recompute the spilled channels exactly on the host.
    smax = int(s_exp.max())
    spilled = perm[:RA][s_exp[perm[:RA]] == smax]
    if spilled.size:
        w_sp = (w_q[spilled].astype(np.float32)
                * np.exp2(s_exp[spilled].astype(np.float32))[:, None])
        out[:, :, spilled] = (
            np.einsum("bti,oi->bto", x.astype(np.float32), w_sp,
                      optimize=True) + bias[spilled])
    return out


# revision 5
# speedup vs baseline: 1.0692x; 1.0010x over previous
"""BitLinear (ternary weight) inference kernel for Trainium2, 8-core SPMD.

Full-input contract: kernel(**inputs) takes the complete tensors and returns
the complete output. The batch dim (B=8) is sharded 1:1 onto the 8
NeuronCores; each core computes y[b] = x[b] @ (w_q * 2^s_exp)^T + bias as a
2048^3 matmul.

Split-precision scheme: output channels are sorted by s_exp (host-side
permutation of w/bias columns, inverted on the output). The 2^s scale is
folded into the ternary weights (+-2^s / 0 are EXACT in fp8e4m3). Channels
with small 2^s (the first RA=1792 after sorting) tolerate fp8 activations:
they use fp8e4m3 x with perf_mode=DoubleRow (2 fp8 weights/cell -> 2
MACs/cell/cycle). The top RB=256 channels (all of s=0) keep fp16
activations at the regular rate for K >= 1024. Measured rel err of this
split on the reference data: ~1.61e-2 (gate 2e-2).

Device schedule per core:
  - All inputs stream on the Sync HWDGE ring (a single queue already fans
    across all 16 SDMA engines, ~400 GB/s): x8/w k-pair-interleaved so the
    contraction frontier advances every ~2.2 us, then the fp16-path data.
    Outputs go on the Scalar ring, bias broadcast-loads on the SWDGE queue.
  - Phase DR (cols 0..1792): 16 row tiles, each a full-K chain of
    DoubleRow matmuls (lhsT [128,2,128] fp8 x pairs, rhs [128,2,*] fp8 w),
    3x512-col psum + 1x256-col psum per tile, two tiles in flight.
  - Phase top-channels (cols 1792..2048): K 0..1024 via fp8 DoubleRow on
    the resident x8 (error sqrt(1024/2048) of full-fp8), K 1024..2048 via
    fp16 x [128,128] stationary x fp8 w [128,256] moving. The unread x16
    quads stay loaded — removing one shifts the SBUF layout and measurably
    slows every matmul (~15%).
  - Epilogue on DVE (psum + bias -> fp16 SBUF), dummy-matmul PE warmup
    sized so the HAM clock gate is at 8/8 when real data lands.
"""
import os

import ml_dtypes
import numpy as np

B, T, IN, OUT = 8, 2048, 2048, 2048
P = 128
NCORES = 8
NF = 512        # psum bank of fp32 / max matmul free dim
RB = 256        # fp16-path columns (largest-s channels, after sorting)
RA = OUT - RB   # DoubleRow fp8 columns (1792)
KP = IN // (2 * P)   # 8 k-pairs
KQ = IN // (4 * P)   # 4 k-quads (x16 DMA granularity)

last_exec_time_ns = None
_CACHE = {}


def _install_prof_shim():
    """Make antenv.axon_hooks importable so trace=True works under axon."""
    import sys
    import types

    if "antenv.axon_hooks" in sys.modules:
        return
    try:
        from trn_agent_boot.trn_boot import _ntff_profile_via_ctypes
    except ImportError:
        return
    hook = _ntff_profile_via_ctypes("/opt/axon/libaxon_pjrt.so")
    mod = types.ModuleType("antenv.axon_hooks")
    mod.get_axon_ntff_profile_hook = lambda: hook
    mod.set_axon_ntff_profile_hook = lambda h: None
    sys.modules["antenv.axon_hooks"] = mod


def _build():
    import concourse.bacc as bacc
    import concourse.bass as bass
    import concourse.mybir as mybir
    from concourse.tile import TileContext

    DR = mybir.MatmulPerfMode.DoubleRow

    nc = bacc.Bacc()
    x8 = nc.dram_tensor("x8", (IN, T), mybir.dt.float8e4, kind="ExternalInput")
    x16 = nc.dram_tensor("x16", (IN, T), mybir.dt.float16, kind="ExternalInput")
    w = nc.dram_tensor("w", (IN, OUT), mybir.dt.float8e4, kind="ExternalInput")
    bias = nc.dram_tensor("bias", (OUT,), mybir.dt.float32, kind="ExternalInput")
    y = nc.dram_tensor("y", (T, OUT), mybir.dt.float16, kind="ExternalOutput")

    TT = T // P     # output row tiles

    with TileContext(nc) as tc:
        with tc.tile_pool(name="wap", bufs=1) as wap, \
             tc.tile_pool(name="x8p", bufs=1) as x8p, \
             tc.tile_pool(name="x16p", bufs=1) as x16p, \
             tc.tile_pool(name="bp", bufs=1) as bp, \
             tc.tile_pool(name="opa", bufs=4) as opa, \
             tc.tile_pool(name="opb", bufs=4) as opb, \
             tc.tile_pool(name="pp", bufs=2, space="PSUM") as pp, \
             tc.tile_pool(name="pg", bufs=2, space="PSUM") as pg:

            bias_t = bp.tile([P, OUT], mybir.dt.float32, tag="bias")
            # Broadcast-load bias (8 KB in DRAM) to all 128 partitions via
            # the SWDGE queue: leading stride-0 AP dim replicates it. Keeps
            # the broadcast off the critical HWDGE input stream.
            bias_ap = bias[:]
            bias_bc = bass.AP(
                tensor=bias_ap.tensor,
                offset=bias_ap.offset,
                ap=[[0, P]] + list(bias_ap.ap),
            )
            nc.gpsimd.dma_start(out=bias_t, in_=bias_bc)

            # HAM pre-warm: dummy matmuls on uninitialized SBUF (the psum
            # target is never read), sized to keep the PE busy until real
            # data arrives so the clock gate reaches 8/8 and stays there.
            warm_sb = bp.tile([P, NF], mybir.dt.float16, tag="warm")
            nc.vector.memset(warm_sb, 0.0)
            warm_ps = pg.tile([P, NF], mybir.dt.float32, tag="psB",
                              name="warmps")
            NWARM = 22
            for i in range(NWARM):
                nc.tensor.matmul(warm_ps, warm_sb[:, :P], warm_sb,
                                 start=(i == 0), stop=(i == NWARM - 1))

            # DRAM views with the k-chunk index explicit.
            x83 = x8.rearrange("(ko p) t -> p ko t", p=P)
            x163 = x16.rearrange("(ko p) t -> p ko t", p=P)
            w3 = w.rearrange("(ko p) o -> p ko o", p=P)

            # Input loads, all on the sync ring in consumption order. The
            # early stream is one token-quarter of each x8 k-pair plus the
            # full (contiguous) w k-pair — ~0.62 MiB per k-pair, matching
            # the two-in-flight row tiles' consumption rate, so the PE
            # tracks the input frontier with ~95% duty. The remaining x8
            # token-quarters, then x16, ride behind.
            x8_t = []
            wa_t = []
            TQ = T // 4
            for kp in range(KP):
                xt = x8p.tile([P, 2, T], mybir.dt.float8e4, tag=f"x8_{kp}")
                wt = wap.tile([P, 2, OUT], mybir.dt.float8e4, tag=f"wa{kp}")
                if kp == 0:
                    # first k-pair: column-quartered w so the first matmul
                    # only waits on ~0.25 MiB
                    nc.sync.dma_start(xt[:, :, :TQ],
                                      x83[:, 0:2, :TQ])
                    for oq in range(4):
                        osl = slice(oq * NF, (oq + 1) * NF)
                        nc.sync.dma_start(wt[:, :, osl], w3[:, 0:2, osl])
                else:
                    nc.sync.dma_start(wt, w3[:, 2 * kp:2 * kp + 2, :])
                    nc.sync.dma_start(xt[:, :, :TQ],
                                      x83[:, 2 * kp:2 * kp + 2, :TQ])
                x8_t.append(xt)
                wa_t.append(wt)
            # remaining token-quarters; quarter 1 is needed by row tile 4
            # at ~30 us, so it goes in one (larger, efficient) load per
            # half-k, ahead of quarters 2-3
            for tq in range(1, 4):
                qsl = slice(tq * TQ, (tq + 1) * TQ)
                for kp in range(KP):
                    nc.sync.dma_start(x8_t[kp][:, :, qsl],
                                      x83[:, 2 * kp:2 * kp + 2, qsl])
            x16_t = []
            for q in range(KQ):
                xt = x16p.tile([P, 4, T], mybir.dt.float16, tag=f"x16_{q}")
                nc.sync.dma_start(xt, x163[:, 4 * q:4 * q + 4, :])
                x16_t.append(xt)

            # Phase DR: 16 full-K row tiles, 1792 cols each.
            for tt in range(TT):
                tsl = slice(tt * P, (tt + 1) * P)
                ps = pp.tile([P, 3 * NF], mybir.dt.float32, tag="psA")
                pq = pg.tile([P, NF], mybir.dt.float32, tag="psB")
                for kp in range(KP):
                    lhsT = x8_t[kp][:, :, tsl]
                    for oc in range(3):
                        nc.tensor.matmul(
                            ps[:, oc * NF:(oc + 1) * NF],
                            lhsT,
                            wa_t[kp][:, :, oc * NF:(oc + 1) * NF],
                            start=(kp == 0), stop=(kp == KP - 1),
                            perf_mode=DR,
                        )
                    nc.tensor.matmul(
                        pq[:, :RA - 3 * NF],
                        lhsT,
                        wa_t[kp][:, :, 3 * NF:RA],
                        start=(kp == 0), stop=(kp == KP - 1),
                        perf_mode=DR,
                    )
                ot = opa.tile([P, RA], mybir.dt.float16, tag="outA")
                nc.vector.tensor_add(ot[:, :3 * NF], ps, bias_t[:, :3 * NF])
                nc.vector.tensor_add(ot[:, 3 * NF:], pq[:, :RA - 3 * NF],
                                     bias_t[:, 3 * NF:RA])
                nc.scalar.dma_start(y[tt * P:(tt + 1) * P, :RA], ot)

            # Phase top-channels (cols RA..OUT): K 0..1024 in fp8 DoubleRow
            # (x8 is resident; error there is ~sqrt(1024/2048) of full-fp8,
            # measured 1.61e-2 combined on the reference data), K
            # 1024..2048 in fp16.
            for tt in range(TT):
                tsl = slice(tt * P, (tt + 1) * P)
                pq = pg.tile([P, NF], mybir.dt.float32, tag="psB")
                for kp in range(4):
                    nc.tensor.matmul(
                        pq[:, :RB],
                        x8_t[kp][:, :, tsl],
                        wa_t[kp][:, :, RA:],
                        start=(kp == 0), stop=False,
                        perf_mode=DR,
                    )
                for k in range(8, IN // P):
                    q, i = divmod(k, 4)
                    nc.tensor.matmul(
                        pq[:, :RB],
                        x16_t[q][:, i, tsl],
                        wa_t[k // 2][:, k % 2, RA:],
                        start=False,
                        stop=(k == IN // P - 1),
                    )
                ot = opb.tile([P, RB], mybir.dt.float16, tag="outB")
                if tt == TT - 1:
                    # last tile: chunk epilogue+store to shorten the tail
                    c = RB // 2
                    for h in range(2):
                        sl = slice(h * c, (h + 1) * c)
                        nc.vector.tensor_add(ot[:, sl], pq[:, sl],
                                             bias_t[:, RA + h * c:
                                                    RA + (h + 1) * c])
                        eng = nc.scalar if h == 0 else nc.sync
                        eng.dma_start(y[tt * P:(tt + 1) * P,
                                        RA + h * c:RA + (h + 1) * c],
                                      ot[:, sl])
                else:
                    nc.vector.tensor_add(ot, pq[:, :RB], bias_t[:, RA:])
                    nc.scalar.dma_start(y[tt * P:(tt + 1) * P, RA:], ot)

    nc.compile()
    return nc


def kernel(x, w_q, s_exp, bias):
    global last_exec_time_ns
    from concourse.bass_utils import run_bass_kernel_spmd

    x = np.asarray(x)
    w_q = np.asarray(w_q)
    s_exp = np.asarray(s_exp)
    bias = np.asarray(bias, dtype=np.float32)
    assert x.shape == (B, T, IN) and w_q.shape == (OUT, IN)

    # Sort output channels by s_exp ascending: the first RA (small 2^s)
    # go through the fp8 DoubleRow path, the top RB through fp16.
    perm = np.argsort(s_exp, kind="stable")

    # Fold the power-of-two per-output-channel scale into the ternary
    # weights: values are +-2^s or 0 with s in [-8, 0], exact in fp8e4m3
    # (2^-8 and 2^-9 are exact subnormals).
    scale = np.exp2(s_exp.astype(np.float32))
    w_scaled_t = (w_q.astype(np.float32) * scale[:, None])[perm].T
    w_fp8 = np.ascontiguousarray(w_scaled_t).astype(ml_dtypes.float8_e4m3fn)
    if not np.array_equal(w_fp8.astype(np.float32), w_scaled_t):
        import warnings
        warnings.warn("scaled ternary weights not exact in fp8e4m3; "
                      "proceeding with rounded weights")
    bias_perm = np.ascontiguousarray(bias[perm].astype(np.float32))
    # Contraction-major layouts for the PE: x^T[b] = [IN, T].
    xT = x.transpose(0, 2, 1)
    xT_f16 = np.ascontiguousarray(xT.astype(np.float16))
    xT_f8 = np.ascontiguousarray(xT.astype(ml_dtypes.float8_e4m3fn))

    nc = _CACHE.get("nc")
    if nc is None:
        nc = _CACHE["nc"] = _build()

    in_maps = [
        {"x8": xT_f8[b], "x16": xT_f16[b], "w": w_fp8, "bias": bias_perm}
        for b in range(B)
    ]

    trace = bool(int(os.environ.get("BITLIN_TRACE", "0")))
    if trace:
        _install_prof_shim()
    res = run_bass_kernel_spmd(nc, in_maps, list(range(NCORES)), trace=trace)
    last_exec_time_ns = res.exec_time_ns
    _CACHE["res"] = res

    y_dev = np.stack([np.asarray(res.results[b]["y"]) for b in range(B)],
                     axis=0).astype(np.float32)
    out = np.empty((B, T, OUT), dtype=np.float32)
    out[:, :, perm] = y_dev

    # Safety net: every channel at the maximum scale 2^smax must take the
    # fp16 path (fp8 error there is ~2.4e-2 > the 2e-2 gate). The device
    # fp16 region holds RB=256 channels; with the spec's s distribution
    # (~228 at smax) it always fits, but if an unusual draw overflows it,
    # recompute the spilled channels exactly on the host.
    smax = int(s_exp.max())
    spilled = perm[:RA][s_exp[perm[:RA]] == smax]
    if spilled.size:
        w_sp = (w_q[spilled].astype(np.float32)
                * np.exp2(s_exp[spilled].astype(np.float32))[:, None])
        out[:, :, spilled] = (
            np.einsum("bti,oi->bto", x.astype(np.float32), w_sp,
                      optimize=True) + bias[spilled])
    return out
